# revision 24
# baseline (speedup 1.0000x reference)
"""Trainium2 Bass kernel for the fused attention-update block.

Math (per sample b; D=1024, INNER=512, H=8, HH=WW=7, S=49):
  val   = oq_v @ Wv + bv
  h     = silu([q-oq, oq] @ Wq1 + bq1) @ Wq2 + bq2
  new_q = sigmoid(h[:D]) * val + q
  new_k = sigmoid(h[D:]) * val + oq                      (output)
  kmap  = LN(new_q; gk, bk_ln) @ Wk + bk                 -> sigk = sigmoid(kmap)
  xv    = LN(x; gx, bx_ln) @ Wx + bx                     [S, INNER]
  value[s, h*64+c] = xv[s, h*64+c] * sigk[h*49+s]
  x_out = value @ Wtv + x                                (output)
  q_out = mean_s(value) @ Wtn + q                        (output)

Strategy: pure data-parallel over batch (1024 -> 8 cores x 128 samples).
Within a core, loop over the 49 spatial positions with position-tiles of
[128 samples, 1024]; everything stays partition-aligned per sample, so the
per-sample gate sigk is read with a strided/broadcast access pattern.
Matmuls run in bf16 (fp32 matmul is 4x slower on the PE); LN statistics,
residual adds and all outputs stay fp32. LN affine params and all biases
are folded on the host into the weight operands (diag(g) @ W folding plus a
K=2 augmented matmul row for the per-feature bias).
"""

import numpy as np
from contextlib import ExitStack

import ml_dtypes

BF = ml_dtypes.bfloat16

B = 1024
NCORES = 8
BS = B // NCORES  # 128 samples per core
S = 49
D = 1024
INNER = 512
H = 8
KD = D // 128    # 8  k-chunks for D contraction
KI = INNER // 128  # 4 k-chunks for INNER contraction
EPS = 1e-5

_CACHED_NC = None
LAST_RESULT = None  # BassKernelResults of the most recent run (for test harness)


def _build_nc():
    import concourse.bass as bass
    import concourse.bacc as bacc
    import concourse.tile as tile
    from concourse import mybir
    from concourse.masks import make_identity

    f32 = mybir.dt.float32
    bf16 = mybir.dt.bfloat16
    AF = mybir.ActivationFunctionType
    ALU = mybir.AluOpType

    nc = bacc.Bacc()

    # ---- DRAM I/O (per-core shard shapes) ----
    x_in = nc.declare_dram_parameter("x", [BS, S, D], f32, isOutput=False)
    q_in = nc.declare_dram_parameter("q", [BS, D], f32, isOutput=False)
    oq_in = nc.declare_dram_parameter("oq", [BS, D], f32, isOutput=False)
    oqv_in = nc.declare_dram_parameter("oqv", [BS, D], f32, isOutput=False)

    wx_in = nc.declare_dram_parameter("wx", [128, KD * INNER], bf16, isOutput=False)
    augx_in = nc.declare_dram_parameter("augx", [2, INNER], bf16, isOutput=False)
    wtv_in = nc.declare_dram_parameter("wtv", [128, KI * D], bf16, isOutput=False)
    wv_in = nc.declare_dram_parameter("wv", [128, KD * D], bf16, isOutput=False)
    wq1_in = nc.declare_dram_parameter("wq1", [128, 16 * 256], bf16, isOutput=False)
    augq1_in = nc.declare_dram_parameter("augq1", [2, 256], bf16, isOutput=False)
    wq2_in = nc.declare_dram_parameter("wq2", [128, 2 * 2048], bf16, isOutput=False)
    bq2r_in = nc.declare_dram_parameter("bq2r", [128, 2048], bf16, isOutput=False)
    bvr_in = nc.declare_dram_parameter("bvr", [128, D], bf16, isOutput=False)
    wk_in = nc.declare_dram_parameter("wk", [128, KD * 392], bf16, isOutput=False)
    augk_in = nc.declare_dram_parameter("augk", [2, 392], bf16, isOutput=False)
    wtn_in = nc.declare_dram_parameter("wtn", [128, KI * D], bf16, isOutput=False)

    xout = nc.declare_dram_parameter("xout", [BS, S, D], f32, isOutput=True)
    qout = nc.declare_dram_parameter("qout", [BS, D], f32, isOutput=True)
    newk = nc.declare_dram_parameter("newk", [BS, D], f32, isOutput=True)

    with tile.TileContext(nc) as tc, ExitStack() as ctx:
        wp = ctx.enter_context(tc.tile_pool(name="weights", bufs=1))
        persist = ctx.enter_context(tc.tile_pool(name="persist", bufs=1))
        xin_pool = ctx.enter_context(tc.tile_pool(name="xin", bufs=6))
        work = ctx.enter_context(tc.tile_pool(name="work", bufs=3))
        stats_pool = ctx.enter_context(tc.tile_pool(name="stats", bufs=6))
        out_pool = ctx.enter_context(tc.tile_pool(name="outs", bufs=2))
        once = ctx.enter_context(tc.tile_pool(name="once", bufs=1))
        # PSUM budget (8 banks): xt 1x2 + vt 1x2 + xv 1x2 + xo 2x1 = 8
        psum = ctx.enter_context(tc.tile_pool(name="psum", bufs=2, space="PSUM"))
        psum2 = ctx.enter_context(tc.tile_pool(name="psum2", bufs=2, space="PSUM"))
        psum_o = ctx.enter_context(tc.tile_pool(name="psum_o", bufs=1, space="PSUM"))

        # ---- constants ----
        ident = wp.tile([128, 128], bf16)
        make_identity(nc, ident)
        ones2 = wp.tile([2, 128], bf16)
        nc.vector.memset(ones2, 0.0)
        nc.vector.memset(ones2[0:1, :], 1.0)
        eps_t = wp.tile([128, 1], f32)
        nc.vector.memset(eps_t, EPS)

        def wtile(name, dram, shape):
            t = wp.tile(list(shape), bf16, tag=f"w_{name}")
            if len(shape) == 3:
                nc.sync.dma_start(out=t, in_=dram.rearrange("p (k n) -> p k n", k=shape[1]))
            else:
                nc.sync.dma_start(out=t, in_=dram[:, :])
            return t

        q_sb = persist.tile([BS, D], f32)
        oq_sb = persist.tile([BS, D], f32)
        oqv_sb = persist.tile([BS, D], f32)
        nc.sync.dma_start(out=q_sb, in_=q_in[:, :])
        nc.sync.dma_start(out=oq_sb, in_=oq_in[:, :])
        nc.sync.dma_start(out=oqv_sb, in_=oqv_in[:, :])
        # per-sample-stage weights first so sigk (which gates every tile)
        # is computable as early as possible; bulk tile-loop weights after.
        wq1 = wtile("wq1", wq1_in, (128, 16, 256))
        augq1 = wtile("augq1", augq1_in, (2, 256))
        wq2 = wtile("wq2", wq2_in, (128, 2, 2048))
        bq2r = wtile("bq2r", bq2r_in, (128, 2048))
        wv = wtile("wv", wv_in, (128, KD, D))
        bvr = wtile("bvr", bvr_in, (128, D))
        wk = wtile("wk", wk_in, (128, KD, 392))
        augk = wtile("augk", augk_in, (2, 392))
        wx = wtile("wx", wx_in, (128, KD, INNER))
        augx = wtile("augx", augx_in, (2, INNER))
        wtv = wtile("wtv", wtv_in, (128, KI, D))
        wtn = wtile("wtn", wtn_in, (128, KI, D))

        import concourse.bass as _bass

        def transpose_to_sbuf(src_bf16, width, copy_engine, tag, split=1):
            """PE-transpose [128, width] bf16 -> PSUM -> copy to SBUF.
            split>1 copies out in pieces so downstream matmuls start sooner."""
            tp = psum.tile([128, width], bf16, tag="xt" if width > 512 else "vt")
            for k in range(width // 128):
                nc.tensor.transpose(tp[:, k * 128:(k + 1) * 128],
                                    src_bf16[:, k * 128:(k + 1) * 128], ident)
            pool = once if tag.startswith("ps_") else work
            nb = 4 if tag == "xnt" else None
            ts = pool.tile([128, width], bf16, tag=tag, name=f"t_{tag}", bufs=nb)
            step = width // split
            for j in range(split):
                copy_engine(out=ts[:, j * step:(j + 1) * step],
                            in_=tp[:, j * step:(j + 1) * step])
            return ts

        def layer_norm_bf16(src_f32, out_tag):
            """bn_stats LN over free dim 1024 -> normalized bf16 tile."""
            v = src_f32.rearrange("p (g d) -> p g d", g=2)
            st = stats_pool.tile([128, 2, 6], f32, tag="bnst")
            for g in range(2):
                nc.vector.bn_stats(out=st[:, g, :], in_=v[:, g, :])
            mv = stats_pool.tile([128, 2], f32, tag="mv")
            nc.vector.bn_aggr(out=mv, in_=st)
            sd = stats_pool.tile([128, 1], f32, tag="sd")
            nc.scalar.activation(out=sd, in_=mv[:, 1:2], func=AF.Sqrt,
                                 bias=eps_t, scale=1.0)
            rstd = stats_pool.tile([128, 1], f32, tag="rstd")
            nc.vector.reciprocal(out=rstd, in_=sd)
            xn = work.tile([128, D], bf16, tag=out_tag)
            nc.vector.tensor_scalar(out=xn, in0=src_f32, scalar1=mv[:, 0:1],
                                    scalar2=rstd[:, 0:1],
                                    op0=ALU.subtract, op1=ALU.mult)
            return xn

        # ================= per-sample stage =================

        # qk = [q - oq, oq] in bf16
        qk1 = once.tile([BS, D], bf16, tag="ps_a")
        nc.vector.tensor_sub(qk1, q_sb, oq_sb)
        qk2 = once.tile([BS, D], bf16, tag="ps_b")
        nc.vector.tensor_copy(qk2, oq_sb)
        qk1T = transpose_to_sbuf(qk1, D, nc.scalar.copy, "ps_at")
        qk2T = transpose_to_sbuf(qk2, D, nc.scalar.copy, "ps_bt")

        # h1 = qk @ Wq1 + bq1  [BS, 256]
        h1_ps = psum2.tile([128, 256], f32, tag="xv")
        for k in range(8):
            nc.tensor.matmul(h1_ps, qk1T[:, k * 128:(k + 1) * 128], wq1[:, k, :],
                             start=(k == 0), stop=False)
        for k in range(8):
            nc.tensor.matmul(h1_ps, qk2T[:, k * 128:(k + 1) * 128], wq1[:, 8 + k, :],
                             start=False, stop=False)
        nc.tensor.matmul(h1_ps, ones2, augq1, start=False, stop=True)

        # silu(h1) = h1 * sigmoid(h1)
        sg1 = once.tile([BS, 256], f32, tag="ps_sg1")
        nc.scalar.activation(out=sg1, in_=h1_ps, func=AF.Sigmoid)
        hsil = once.tile([BS, 256], bf16, tag="ps_hsil")
        nc.vector.tensor_mul(hsil, h1_ps, sg1)
        hT = transpose_to_sbuf(hsil, 256, nc.scalar.copy, "ps_ht")

        # h2 = silu(h1) @ Wq2 (+ bq2 on evict), split into query/key halves
        sqk = []
        for half in range(2):
            h2_h = [psum_o.tile([128, 512], f32, tag=f"xo{n}", name=f"h2_h{n}") for n in range(2)]
            for n in range(2):
                for k in range(2):
                    nc.tensor.matmul(
                        h2_h[n],
                        hT[:, k * 128:(k + 1) * 128],
                        wq2[:, k, half * 1024 + n * 512: half * 1024 + (n + 1) * 512],
                        start=(k == 0), stop=(k == 1))
            th = once.tile([BS, D], f32, tag=f"ps_th{half}")
            for n in range(2):
                nc.vector.tensor_add(th[:, n * 512:(n + 1) * 512], h2_h[n],
                                     bq2r[:, half * 1024 + n * 512:half * 1024 + (n + 1) * 512])
            sig = persist.tile([BS, D], f32, tag=f"ps_sig{half}")
            nc.scalar.activation(out=sig, in_=th, func=AF.Sigmoid)
            sqk.append(sig)

        # val = oq_v @ Wv + bv
        oqv_bf = once.tile([BS, D], bf16, tag="ps_a")
        nc.vector.tensor_copy(oqv_bf, oqv_sb)
        oqvT = transpose_to_sbuf(oqv_bf, D, nc.scalar.copy, "ps_at")
        val_h = [psum_o.tile([128, 512], f32, tag=f"xo{n}", name=f"val_h{n}") for n in range(2)]
        for n in range(2):
            for k in range(KD):
                nc.tensor.matmul(val_h[n],
                                 oqvT[:, k * 128:(k + 1) * 128],
                                 wv[:, k, n * 512:(n + 1) * 512],
                                 start=(k == 0), stop=(k == KD - 1))
        val_sb = persist.tile([BS, D], f32)
        for n in range(2):
            nc.vector.tensor_add(val_sb[:, n * 512:(n + 1) * 512], val_h[n],
                                 bvr[:, n * 512:(n + 1) * 512])

        # new_q / new_k
        nq = persist.tile([BS, D], f32)
        nc.vector.tensor_mul(nq, sqk[0], val_sb)
        nc.vector.tensor_add(nq, nq, q_sb)
        nk = persist.tile([BS, D], f32)
        nc.vector.tensor_mul(nk, sqk[1], val_sb)
        nc.vector.tensor_add(nk, nk, oq_sb)
        nc.scalar.dma_start(out=newk[:, :], in_=nk)

        # kmap = LN(new_q) @ Wk' + c0k ; sigk = sigmoid(kmap)
        zq = layer_norm_bf16(nq, "ps_a")
        zqT = transpose_to_sbuf(zq, D, nc.scalar.copy, "ps_at")
        kmap_ps = psum2.tile([128, 392], f32, tag="xv")
        for k in range(KD):
            nc.tensor.matmul(kmap_ps, zqT[:, k * 128:(k + 1) * 128], wk[:, k, :],
                             start=(k == 0), stop=False)
        nc.tensor.matmul(kmap_ps, ones2, augk, start=False, stop=True)
        sigk = persist.tile([BS, H * S], bf16)
        nc.scalar.activation(out=sigk, in_=kmap_ps, func=AF.Sigmoid)

        # running sum of value over positions (for q_out)
        vacc = persist.tile([BS, INNER], f32)
        nc.vector.memset(vacc, 0.0)

        # ================= position-tile loop =================
        # Tiles are processed in pairs with matmul work batched per pair:
        # the PE sees long uninterrupted MATMUL bursts (keeps the HAM clock
        # gate at 8/8) instead of alternating transpose/matmul every tile.
        import os as _os
        _ntiles = int(_os.environ.get("K_NTILES", S))

        def tile_phase1(s):
            x_sb = xin_pool.tile([BS, D], f32, tag="x", name=f"x_{s}")
            nc.sync.dma_start(out=x_sb, in_=x_in[:, s, :])
            xn = layer_norm_bf16(x_sb, "xn")
            xnT = transpose_to_sbuf(xn, D, nc.scalar.copy, "xnt", split=2)
            return x_sb, xnT

        def tile_tox(xnT):
            xv_ps = psum2.tile([128, INNER], f32, tag="xv", name="xv_ps")
            for k in range(KD):
                nc.tensor.matmul(xv_ps, xnT[:, k * 128:(k + 1) * 128], wx[:, k, :],
                                 start=(k == 0), stop=False)
            nc.tensor.matmul(xv_ps, ones2, augx, start=False, stop=True)
            return xv_ps

        def tile_gate(s, xv_ps):
            gate_ap = _bass.AP(tensor=sigk.tensor, offset=sigk.offset + s,
                               ap=[sigk.ap[0], [S, H], [0, 64]])
            value = work.tile([BS, INNER], bf16, tag="value", name=f"value_{s}")
            nc.vector.tensor_mul(value, xv_ps, gate_ap)
            nc.gpsimd.tensor_add(vacc, vacc, value)
            return value

        def tile_tov(s, x_sb, vT):
            xo_h = [psum_o.tile([128, 512], f32, tag=f"xo{n}", name=f"xo_h{n}") for n in range(2)]
            xo_sb = out_pool.tile([BS, D], f32, tag="xo_sb", name=f"xo_sb_{s}")
            for n in range(2):
                for k in range(KI):
                    nc.tensor.matmul(xo_h[n],
                                     vT[:, k * 128:(k + 1) * 128],
                                     wtv[:, k, n * 512:(n + 1) * 512],
                                     start=(k == 0), stop=(k == KI - 1))
                nc.scalar.copy(out=xo_sb[:, n * 512:(n + 1) * 512], in_=xo_h[n])
            nc.gpsimd.tensor_add(xo_sb, xo_sb, x_sb)
            nc.scalar.dma_start(out=xout[:, s, :], in_=xo_sb)

        def tile_phase2(pair, p1):
            xvs = [tile_tox(xnT) for (_, xnT) in p1]
            vals = [tile_gate(s, xv) for s, xv in zip(pair, xvs)]
            vTs = [transpose_to_sbuf(v, INNER, nc.vector.tensor_copy, "vt")
                   for v in vals]
            for (x_sb, _), s, vT in zip(p1, pair, vTs):
                tile_tov(s, x_sb, vT)

        # one pair of lookahead: phase1(p+1) is emitted before phase2(p) so
        # the next pair's LN/transposes are scheduled ahead of this pair's
        # evictions on the ACT/DVE queues.
        pending = None
        for s0 in range(0, _ntiles, 2):
            pair = [s for s in (s0, s0 + 1) if s < _ntiles]
            p1 = [tile_phase1(s) for s in pair]
            if pending is not None:
                tile_phase2(*pending)
            pending = (pair, p1)
        if pending is not None:
            tile_phase2(*pending)

        # ================= q_out tail =================
        vacc_bf = work.tile([BS, INNER], bf16, tag="value")
        nc.vector.tensor_copy(vacc_bf, vacc)
        vaccT = transpose_to_sbuf(vacc_bf, INNER, nc.vector.tensor_copy, "vt")
        qo_h = [psum_o.tile([128, 512], f32, tag=f"xo{n}", name=f"qo_h{n}") for n in range(2)]
        qo_sb = persist.tile([BS, D], f32, tag="qo_sb")
        for n in range(2):
            for k in range(KI):
                nc.tensor.matmul(qo_h[n],
                                 vaccT[:, k * 128:(k + 1) * 128],
                                 wtn[:, k, n * 512:(n + 1) * 512],
                                 start=(k == 0), stop=(k == KI - 1))
            nc.vector.tensor_add(qo_sb[:, n * 512:(n + 1) * 512], qo_h[n],
                                 q_sb[:, n * 512:(n + 1) * 512])
        nc.scalar.dma_start(out=qout[:, :], in_=qo_sb)

    nc.finalize()
    return nc


def _pack(w, kchunks):
    """[K*128, N] -> [128, kchunks*N] bf16 with k-chunk-major free dim."""
    w = np.asarray(w, np.float32)
    n = w.shape[1]
    return np.ascontiguousarray(
        w.astype(BF).reshape(kchunks, 128, n).transpose(1, 0, 2)
    ).reshape(128, kchunks * n)


def kernel(**inputs):
    global _CACHED_NC, LAST_RESULT
    from concourse.bass_utils import run_bass_kernel_spmd

    f = lambda name: np.asarray(inputs[name], np.float32)
    x = f("x")
    q = f("q").reshape(B, D)
    oq = f("oq").reshape(B, D)
    oqv = f("oq_v").reshape(B, D)

    gx, bxln, Wx, bx = f("gx"), f("bx_ln"), f("Wx"), f("bx")
    gk, bkln, Wk, bk = f("gk"), f("bk_ln"), f("Wk"), f("bk")
    Wq1, bq1 = f("Wq1"), f("bq1")
    Wq2, bq2 = f("Wq2"), f("bq2")
    Wv, bv = f("Wv"), f("bv")
    Wtv, Wtn = f("Wtv"), f("Wtn")

    wxf = gx[:, None] * Wx
    c0x = bxln @ Wx + bx
    wkf = gk[:, None] * Wk
    c0k = bkln @ Wk + bk

    def aug(row0, n):
        a = np.zeros((2, n), dtype=BF)
        a[0] = row0.astype(BF)
        return a

    weights = dict(
        wx=_pack(wxf, KD), augx=aug(c0x, INNER),
        wtv=_pack(Wtv, KI), wv=_pack(Wv, KD),
        wq1=_pack(Wq1, 16), augq1=aug(bq1, 256),
        wq2=_pack(Wq2, 2),
        bq2r=np.ascontiguousarray(np.tile(bq2.astype(BF), (128, 1))),
        bvr=np.ascontiguousarray(np.tile(bv.astype(BF), (128, 1))),
        wk=_pack(wkf, KD), augk=aug(c0k, 392),
        wtn=_pack(Wtn / np.float32(S), KI),
    )

    if _CACHED_NC is None:
        _CACHED_NC = _build_nc()
    nc = _CACHED_NC

    in_maps = []
    for i in range(NCORES):
        sl = slice(i * BS, (i + 1) * BS)
        in_maps.append(dict(
            x=np.ascontiguousarray(x[sl]),
            q=np.ascontiguousarray(q[sl]),
            oq=np.ascontiguousarray(oq[sl]),
            oqv=np.ascontiguousarray(oqv[sl]),
            **weights,
        ))

    LAST_RESULT = run_bass_kernel_spmd(nc, in_maps, list(range(NCORES)))
    res = LAST_RESULT.results

    x_out = np.concatenate([r["xout"] for r in res], axis=0)
    q_out = np.concatenate([r["qout"] for r in res], axis=0).reshape(B, 1, D)
    new_k = np.concatenate([r["newk"] for r in res], axis=0).reshape(B, 1, D)
    return (np.asarray(x_out, np.float32), np.asarray(q_out, np.float32),
            np.asarray(new_k, np.float32))


# revision 25
# speedup vs baseline: 1.1015x; 1.1015x over previous
"""Trainium2 Bass kernel for the fused attention-update block.

Math (per sample b; D=1024, INNER=512, H=8, HH=WW=7, S=49):
  val   = oq_v @ Wv + bv
  h     = silu([q-oq, oq] @ Wq1 + bq1) @ Wq2 + bq2
  new_q = sigmoid(h[:D]) * val + q
  new_k = sigmoid(h[D:]) * val + oq                      (output)
  kmap  = LN(new_q; gk, bk_ln) @ Wk + bk                 -> sigk = sigmoid(kmap)
  xv    = LN(x; gx, bx_ln) @ Wx + bx                     [S, INNER]
  value[s, h*64+c] = xv[s, h*64+c] * sigk[h*49+s]
  x_out = value @ Wtv + x                                (output)
  q_out = mean_s(value) @ Wtn + q                        (output)

Strategy: pure data-parallel over batch (1024 -> 8 cores x 128 samples).
Within a core, loop over the 49 spatial positions with position-tiles of
[128 samples, 1024]; everything stays partition-aligned per sample, so the
per-sample gate sigk is read with a strided/broadcast access pattern.
Matmuls run in bf16 (fp32 matmul is 4x slower on the PE); LN statistics,
residual adds and all outputs stay fp32. LN affine params and all biases
are folded on the host into the weight operands (diag(g) @ W folding plus a
K=2 augmented matmul row for the per-feature bias).
"""

import numpy as np
from contextlib import ExitStack

import ml_dtypes

BF = ml_dtypes.bfloat16

B = 1024
NCORES = 8
BS = B // NCORES  # 128 samples per core
S = 49
D = 1024
INNER = 512
H = 8
KD = D // 128    # 8  k-chunks for D contraction
KI = INNER // 128  # 4 k-chunks for INNER contraction
EPS = 1e-5

_CACHED_NC = None
LAST_RESULT = None  # BassKernelResults of the most recent run (for test harness)


def _build_nc():
    import concourse.bass as bass
    import concourse.bacc as bacc
    import concourse.tile as tile
    from concourse import mybir
    from concourse.masks import make_identity

    f32 = mybir.dt.float32
    bf16 = mybir.dt.bfloat16
    AF = mybir.ActivationFunctionType
    ALU = mybir.AluOpType

    nc = bacc.Bacc()

    # ---- DRAM I/O (per-core shard shapes) ----
    x_in = nc.declare_dram_parameter("x", [BS, S, D], f32, isOutput=False)
    q_in = nc.declare_dram_parameter("q", [BS, D], f32, isOutput=False)
    oq_in = nc.declare_dram_parameter("oq", [BS, D], f32, isOutput=False)
    oqv_in = nc.declare_dram_parameter("oqv", [BS, D], f32, isOutput=False)

    wx_in = nc.declare_dram_parameter("wx", [128, KD * INNER], bf16, isOutput=False)
    augx_in = nc.declare_dram_parameter("augx", [2, INNER], bf16, isOutput=False)
    wtv_in = nc.declare_dram_parameter("wtv", [128, KI * D], bf16, isOutput=False)
    wv_in = nc.declare_dram_parameter("wv", [128, KD * D], bf16, isOutput=False)
    wq1_in = nc.declare_dram_parameter("wq1", [128, 16 * 256], bf16, isOutput=False)
    augq1_in = nc.declare_dram_parameter("augq1", [2, 256], bf16, isOutput=False)
    wq2_in = nc.declare_dram_parameter("wq2", [128, 2 * 2048], bf16, isOutput=False)
    bq2r_in = nc.declare_dram_parameter("bq2r", [128, 2048], bf16, isOutput=False)
    bvr_in = nc.declare_dram_parameter("bvr", [128, D], bf16, isOutput=False)
    wk_in = nc.declare_dram_parameter("wk", [128, KD * 392], bf16, isOutput=False)
    augk_in = nc.declare_dram_parameter("augk", [2, 392], bf16, isOutput=False)
    wtn_in = nc.declare_dram_parameter("wtn", [128, KI * D], bf16, isOutput=False)

    xout = nc.declare_dram_parameter("xout", [BS, S, D], f32, isOutput=True)
    qout = nc.declare_dram_parameter("qout", [BS, D], f32, isOutput=True)
    newk = nc.declare_dram_parameter("newk", [BS, D], f32, isOutput=True)

    with tile.TileContext(nc) as tc, ExitStack() as ctx:
        wp = ctx.enter_context(tc.tile_pool(name="weights", bufs=1))
        persist = ctx.enter_context(tc.tile_pool(name="persist", bufs=1))
        xin_pool = ctx.enter_context(tc.tile_pool(name="xin", bufs=8))
        work = ctx.enter_context(tc.tile_pool(name="work", bufs=3))
        stats_pool = ctx.enter_context(tc.tile_pool(name="stats", bufs=6))
        out_pool = ctx.enter_context(tc.tile_pool(name="outs", bufs=2))
        once = ctx.enter_context(tc.tile_pool(name="once", bufs=1))
        # PSUM budget (8 banks): xt 1x2 + vt 1x2 + xv 1x2 + xo 2x1 = 8
        psum = ctx.enter_context(tc.tile_pool(name="psum", bufs=2, space="PSUM"))
        psum2 = ctx.enter_context(tc.tile_pool(name="psum2", bufs=2, space="PSUM"))
        psum_o = ctx.enter_context(tc.tile_pool(name="psum_o", bufs=1, space="PSUM"))

        # ---- constants ----
        ident = wp.tile([128, 128], bf16)
        make_identity(nc, ident)
        ones2 = wp.tile([2, 128], bf16)
        nc.vector.memset(ones2, 0.0)
        nc.vector.memset(ones2[0:1, :], 1.0)
        eps_t = wp.tile([128, 1], f32)
        nc.vector.memset(eps_t, EPS)

        def wtile(name, dram, shape):
            t = wp.tile(list(shape), bf16, tag=f"w_{name}")
            if len(shape) == 3:
                nc.sync.dma_start(out=t, in_=dram.rearrange("p (k n) -> p k n", k=shape[1]))
            else:
                nc.sync.dma_start(out=t, in_=dram[:, :])
            return t

        q_sb = persist.tile([BS, D], f32)
        oq_sb = persist.tile([BS, D], f32)
        oqv_sb = persist.tile([BS, D], f32)
        nc.sync.dma_start(out=q_sb, in_=q_in[:, :])
        nc.sync.dma_start(out=oq_sb, in_=oq_in[:, :])
        nc.sync.dma_start(out=oqv_sb, in_=oqv_in[:, :])
        # per-sample-stage weights first so sigk (which gates every tile)
        # is computable as early as possible; bulk tile-loop weights after.
        wq1 = wtile("wq1", wq1_in, (128, 16, 256))
        augq1 = wtile("augq1", augq1_in, (2, 256))
        wq2 = wtile("wq2", wq2_in, (128, 2, 2048))
        bq2r = wtile("bq2r", bq2r_in, (128, 2048))
        wv = wtile("wv", wv_in, (128, KD, D))
        bvr = wtile("bvr", bvr_in, (128, D))
        wk = wtile("wk", wk_in, (128, KD, 392))
        augk = wtile("augk", augk_in, (2, 392))
        wx = wtile("wx", wx_in, (128, KD, INNER))
        augx = wtile("augx", augx_in, (2, INNER))
        wtv = wtile("wtv", wtv_in, (128, KI, D))
        wtn = wtile("wtn", wtn_in, (128, KI, D))

        import concourse.bass as _bass

        def transpose_to_sbuf(src_bf16, width, copy_engine, tag, split=1):
            """PE-transpose [128, width] bf16 -> PSUM -> copy to SBUF.
            split>1 copies out in pieces so downstream matmuls start sooner."""
            tp = psum.tile([128, width], bf16, tag="xt" if width > 512 else "vt")
            for k in range(width // 128):
                nc.tensor.transpose(tp[:, k * 128:(k + 1) * 128],
                                    src_bf16[:, k * 128:(k + 1) * 128], ident)
            pool = once if tag.startswith("ps_") else work
            nb = 6 if tag == "xnt" else None
            ts = pool.tile([128, width], bf16, tag=tag, name=f"t_{tag}", bufs=nb)
            step = width // split
            for j in range(split):
                copy_engine(out=ts[:, j * step:(j + 1) * step],
                            in_=tp[:, j * step:(j + 1) * step])
            return ts

        def layer_norm_bf16(src_f32, out_tag):
            """bn_stats LN over free dim 1024 -> normalized bf16 tile."""
            v = src_f32.rearrange("p (g d) -> p g d", g=2)
            st = stats_pool.tile([128, 2, 6], f32, tag="bnst")
            for g in range(2):
                nc.vector.bn_stats(out=st[:, g, :], in_=v[:, g, :])
            mv = stats_pool.tile([128, 2], f32, tag="mv")
            nc.vector.bn_aggr(out=mv, in_=st)
            sd = stats_pool.tile([128, 1], f32, tag="sd")
            nc.scalar.activation(out=sd, in_=mv[:, 1:2], func=AF.Sqrt,
                                 bias=eps_t, scale=1.0)
            rstd = stats_pool.tile([128, 1], f32, tag="rstd")
            nc.vector.reciprocal(out=rstd, in_=sd)
            xn = work.tile([128, D], bf16, tag=out_tag)
            nc.vector.tensor_scalar(out=xn, in0=src_f32, scalar1=mv[:, 0:1],
                                    scalar2=rstd[:, 0:1],
                                    op0=ALU.subtract, op1=ALU.mult)
            return xn

        # ================= per-sample stage =================

        # qk = [q - oq, oq] in bf16
        qk1 = once.tile([BS, D], bf16, tag="ps_a")
        nc.vector.tensor_sub(qk1, q_sb, oq_sb)
        qk2 = once.tile([BS, D], bf16, tag="ps_b")
        nc.vector.tensor_copy(qk2, oq_sb)
        qk1T = transpose_to_sbuf(qk1, D, nc.scalar.copy, "ps_at")
        qk2T = transpose_to_sbuf(qk2, D, nc.scalar.copy, "ps_bt")

        # h1 = qk @ Wq1 + bq1  [BS, 256]
        h1_ps = psum2.tile([128, 256], f32, tag="xv")
        for k in range(8):
            nc.tensor.matmul(h1_ps, qk1T[:, k * 128:(k + 1) * 128], wq1[:, k, :],
                             start=(k == 0), stop=False)
        for k in range(8):
            nc.tensor.matmul(h1_ps, qk2T[:, k * 128:(k + 1) * 128], wq1[:, 8 + k, :],
                             start=False, stop=False)
        nc.tensor.matmul(h1_ps, ones2, augq1, start=False, stop=True)

        # silu(h1) = h1 * sigmoid(h1)
        sg1 = once.tile([BS, 256], f32, tag="ps_sg1")
        nc.scalar.activation(out=sg1, in_=h1_ps, func=AF.Sigmoid)
        hsil = once.tile([BS, 256], bf16, tag="ps_hsil")
        nc.vector.tensor_mul(hsil, h1_ps, sg1)
        hT = transpose_to_sbuf(hsil, 256, nc.scalar.copy, "ps_ht")

        # h2 = silu(h1) @ Wq2 (+ bq2 on evict), split into query/key halves
        sqk = []
        for half in range(2):
            h2_h = [psum_o.tile([128, 512], f32, tag=f"xo{n}", name=f"h2_h{n}") for n in range(2)]
            for n in range(2):
                for k in range(2):
                    nc.tensor.matmul(
                        h2_h[n],
                        hT[:, k * 128:(k + 1) * 128],
                        wq2[:, k, half * 1024 + n * 512: half * 1024 + (n + 1) * 512],
                        start=(k == 0), stop=(k == 1))
            th = once.tile([BS, D], f32, tag=f"ps_th{half}")
            for n in range(2):
                nc.vector.tensor_add(th[:, n * 512:(n + 1) * 512], h2_h[n],
                                     bq2r[:, half * 1024 + n * 512:half * 1024 + (n + 1) * 512])
            sig = persist.tile([BS, D], f32, tag=f"ps_sig{half}")
            nc.scalar.activation(out=sig, in_=th, func=AF.Sigmoid)
            sqk.append(sig)

        # val = oq_v @ Wv + bv
        oqv_bf = once.tile([BS, D], bf16, tag="ps_a")
        nc.vector.tensor_copy(oqv_bf, oqv_sb)
        oqvT = transpose_to_sbuf(oqv_bf, D, nc.scalar.copy, "ps_at")
        val_h = [psum_o.tile([128, 512], f32, tag=f"xo{n}", name=f"val_h{n}") for n in range(2)]
        for n in range(2):
            for k in range(KD):
                nc.tensor.matmul(val_h[n],
                                 oqvT[:, k * 128:(k + 1) * 128],
                                 wv[:, k, n * 512:(n + 1) * 512],
                                 start=(k == 0), stop=(k == KD - 1))
        val_sb = persist.tile([BS, D], f32)
        for n in range(2):
            nc.vector.tensor_add(val_sb[:, n * 512:(n + 1) * 512], val_h[n],
                                 bvr[:, n * 512:(n + 1) * 512])

        # new_q / new_k
        nq = persist.tile([BS, D], f32)
        nc.vector.tensor_mul(nq, sqk[0], val_sb)
        nc.vector.tensor_add(nq, nq, q_sb)
        nk = persist.tile([BS, D], f32)
        nc.vector.tensor_mul(nk, sqk[1], val_sb)
        nc.vector.tensor_add(nk, nk, oq_sb)
        nc.scalar.dma_start(out=newk[:, :], in_=nk)

        # kmap = LN(new_q) @ Wk' + c0k ; sigk = sigmoid(kmap)
        zq = layer_norm_bf16(nq, "ps_a")
        zqT = transpose_to_sbuf(zq, D, nc.scalar.copy, "ps_at")
        kmap_ps = psum2.tile([128, 392], f32, tag="xv")
        for k in range(KD):
            nc.tensor.matmul(kmap_ps, zqT[:, k * 128:(k + 1) * 128], wk[:, k, :],
                             start=(k == 0), stop=False)
        nc.tensor.matmul(kmap_ps, ones2, augk, start=False, stop=True)
        sigk = persist.tile([BS, H * S], bf16)
        nc.scalar.activation(out=sigk, in_=kmap_ps, func=AF.Sigmoid)

        # running sum of value over positions (for q_out)
        vacc = persist.tile([BS, INNER], f32)
        nc.vector.memset(vacc, 0.0)

        # ================= position-tile loop =================
        # Tiles are processed in pairs with matmul work batched per pair:
        # the PE sees long uninterrupted MATMUL bursts (keeps the HAM clock
        # gate at 8/8) instead of alternating transpose/matmul every tile.
        import os as _os
        _ntiles = int(_os.environ.get("K_NTILES", S))

        def tile_phase1(s):
            x_sb = xin_pool.tile([BS, D], f32, tag="x", name=f"x_{s}")
            nc.sync.dma_start(out=x_sb, in_=x_in[:, s, :])
            xn = layer_norm_bf16(x_sb, "xn")
            xnT = transpose_to_sbuf(xn, D, nc.scalar.copy, "xnt", split=2)
            return x_sb, xnT

        def tile_tox(xnT):
            xv_ps = psum2.tile([128, INNER], f32, tag="xv", name="xv_ps")
            for k in range(KD):
                nc.tensor.matmul(xv_ps, xnT[:, k * 128:(k + 1) * 128], wx[:, k, :],
                                 start=(k == 0), stop=False)
            nc.tensor.matmul(xv_ps, ones2, augx, start=False, stop=True)
            return xv_ps

        def tile_gate(s, xv_ps):
            gate_ap = _bass.AP(tensor=sigk.tensor, offset=sigk.offset + s,
                               ap=[sigk.ap[0], [S, H], [0, 64]])
            value = work.tile([BS, INNER], bf16, tag="value", name=f"value_{s}")
            nc.vector.tensor_mul(value, xv_ps, gate_ap)
            nc.gpsimd.tensor_add(vacc, vacc, value)
            return value

        def tile_tov(s, x_sb, vT):
            xo_h = [psum_o.tile([128, 512], f32, tag=f"xo{n}", name=f"xo_h{n}") for n in range(2)]
            xo_sb = out_pool.tile([BS, D], f32, tag="xo_sb", name=f"xo_sb_{s}")
            for n in range(2):
                for k in range(KI):
                    nc.tensor.matmul(xo_h[n],
                                     vT[:, k * 128:(k + 1) * 128],
                                     wtv[:, k, n * 512:(n + 1) * 512],
                                     start=(k == 0), stop=(k == KI - 1))
                nc.scalar.copy(out=xo_sb[:, n * 512:(n + 1) * 512], in_=xo_h[n])
            nc.gpsimd.tensor_add(xo_sb, xo_sb, x_sb)
            nc.scalar.dma_start(out=xout[:, s, :], in_=xo_sb)

        def tile_phase2(pair, p1):
            xvs = [tile_tox(xnT) for (_, xnT) in p1]
            vals = [tile_gate(s, xv) for s, xv in zip(pair, xvs)]
            vTs = [transpose_to_sbuf(v, INNER, nc.vector.tensor_copy, "vt")
                   for v in vals]
            for (x_sb, _), s, vT in zip(p1, pair, vTs):
                tile_tov(s, x_sb, vT)

        # one pair of lookahead: phase1(p+1) is emitted before phase2(p) so
        # the next pair's LN/transposes are scheduled ahead of this pair's
        # evictions on the ACT/DVE queues.
        pending = None
        for s0 in range(0, _ntiles, 2):
            pair = [s for s in (s0, s0 + 1) if s < _ntiles]
            p1 = [tile_phase1(s) for s in pair]
            if pending is not None:
                tile_phase2(*pending)
            pending = (pair, p1)
        if pending is not None:
            tile_phase2(*pending)

        # ================= q_out tail =================
        vacc_bf = work.tile([BS, INNER], bf16, tag="value")
        nc.vector.tensor_copy(vacc_bf, vacc)
        vaccT = transpose_to_sbuf(vacc_bf, INNER, nc.vector.tensor_copy, "vt")
        qo_h = [psum_o.tile([128, 512], f32, tag=f"xo{n}", name=f"qo_h{n}") for n in range(2)]
        qo_sb = persist.tile([BS, D], f32, tag="qo_sb")
        for n in range(2):
            for k in range(KI):
                nc.tensor.matmul(qo_h[n],
                                 vaccT[:, k * 128:(k + 1) * 128],
                                 wtn[:, k, n * 512:(n + 1) * 512],
                                 start=(k == 0), stop=(k == KI - 1))
            nc.vector.tensor_add(qo_sb[:, n * 512:(n + 1) * 512], qo_h[n],
                                 q_sb[:, n * 512:(n + 1) * 512])
        nc.scalar.dma_start(out=qout[:, :], in_=qo_sb)

    nc.finalize()
    return nc


def _pack(w, kchunks):
    """[K*128, N] -> [128, kchunks*N] bf16 with k-chunk-major free dim."""
    w = np.asarray(w, np.float32)
    n = w.shape[1]
    return np.ascontiguousarray(
        w.astype(BF).reshape(kchunks, 128, n).transpose(1, 0, 2)
    ).reshape(128, kchunks * n)


def kernel(**inputs):
    global _CACHED_NC, LAST_RESULT
    from concourse.bass_utils import run_bass_kernel_spmd

    f = lambda name: np.asarray(inputs[name], np.float32)
    x = f("x")
    q = f("q").reshape(B, D)
    oq = f("oq").reshape(B, D)
    oqv = f("oq_v").reshape(B, D)

    gx, bxln, Wx, bx = f("gx"), f("bx_ln"), f("Wx"), f("bx")
    gk, bkln, Wk, bk = f("gk"), f("bk_ln"), f("Wk"), f("bk")
    Wq1, bq1 = f("Wq1"), f("bq1")
    Wq2, bq2 = f("Wq2"), f("bq2")
    Wv, bv = f("Wv"), f("bv")
    Wtv, Wtn = f("Wtv"), f("Wtn")

    wxf = gx[:, None] * Wx
    c0x = bxln @ Wx + bx
    wkf = gk[:, None] * Wk
    c0k = bkln @ Wk + bk

    def aug(row0, n):
        a = np.zeros((2, n), dtype=BF)
        a[0] = row0.astype(BF)
        return a

    weights = dict(
        wx=_pack(wxf, KD), augx=aug(c0x, INNER),
        wtv=_pack(Wtv, KI), wv=_pack(Wv, KD),
        wq1=_pack(Wq1, 16), augq1=aug(bq1, 256),
        wq2=_pack(Wq2, 2),
        bq2r=np.ascontiguousarray(np.tile(bq2.astype(BF), (128, 1))),
        bvr=np.ascontiguousarray(np.tile(bv.astype(BF), (128, 1))),
        wk=_pack(wkf, KD), augk=aug(c0k, 392),
        wtn=_pack(Wtn / np.float32(S), KI),
    )

    if _CACHED_NC is None:
        _CACHED_NC = _build_nc()
    nc = _CACHED_NC

    in_maps = []
    for i in range(NCORES):
        sl = slice(i * BS, (i + 1) * BS)
        in_maps.append(dict(
            x=np.ascontiguousarray(x[sl]),
            q=np.ascontiguousarray(q[sl]),
            oq=np.ascontiguousarray(oq[sl]),
            oqv=np.ascontiguousarray(oqv[sl]),
            **weights,
        ))

    LAST_RESULT = run_bass_kernel_spmd(nc, in_maps, list(range(NCORES)))
    res = LAST_RESULT.results

    x_out = np.concatenate([r["xout"] for r in res], axis=0)
    q_out = np.concatenate([r["qout"] for r in res], axis=0).reshape(B, 1, D)
    new_k = np.concatenate([r["newk"] for r in res], axis=0).reshape(B, 1, D)
    return (np.asarray(x_out, np.float32), np.asarray(q_out, np.float32),
            np.asarray(new_k, np.float32))


# revision 26
# speedup vs baseline: 1.1290x; 1.0250x over previous
"""Trainium2 Bass kernel for the fused attention-update block.

Math (per sample b; D=1024, INNER=512, H=8, HH=WW=7, S=49):
  val   = oq_v @ Wv + bv
  h     = silu([q-oq, oq] @ Wq1 + bq1) @ Wq2 + bq2
  new_q = sigmoid(h[:D]) * val + q
  new_k = sigmoid(h[D:]) * val + oq                      (output)
  kmap  = LN(new_q; gk, bk_ln) @ Wk + bk                 -> sigk = sigmoid(kmap)
  xv    = LN(x; gx, bx_ln) @ Wx + bx                     [S, INNER]
  value[s, h*64+c] = xv[s, h*64+c] * sigk[h*49+s]
  x_out = value @ Wtv + x                                (output)
  q_out = mean_s(value) @ Wtn + q                        (output)

Strategy: pure data-parallel over batch (1024 -> 8 cores x 128 samples).
Within a core, loop over the 49 spatial positions with position-tiles of
[128 samples, 1024]; everything stays partition-aligned per sample, so the
per-sample gate sigk is read with a strided/broadcast access pattern.
Matmuls run in bf16 (fp32 matmul is 4x slower on the PE); LN statistics,
residual adds and all outputs stay fp32. LN affine params and all biases
are folded on the host into the weight operands (diag(g) @ W folding plus a
K=2 augmented matmul row for the per-feature bias).
"""

import numpy as np
from contextlib import ExitStack

import ml_dtypes

BF = ml_dtypes.bfloat16

B = 1024
NCORES = 8
BS = B // NCORES  # 128 samples per core
S = 49
D = 1024
INNER = 512
H = 8
KD = D // 128    # 8  k-chunks for D contraction
KI = INNER // 128  # 4 k-chunks for INNER contraction
EPS = 1e-5

_CACHED_NC = None
LAST_RESULT = None  # BassKernelResults of the most recent run (for test harness)


def _build_nc():
    import concourse.bass as bass
    import concourse.bacc as bacc
    import concourse.tile as tile
    from concourse import mybir
    from concourse.masks import make_identity

    f32 = mybir.dt.float32
    bf16 = mybir.dt.bfloat16
    AF = mybir.ActivationFunctionType
    ALU = mybir.AluOpType

    nc = bacc.Bacc()

    # ---- DRAM I/O (per-core shard shapes) ----
    x_in = nc.declare_dram_parameter("x", [BS, S, D], f32, isOutput=False)
    q_in = nc.declare_dram_parameter("q", [BS, D], f32, isOutput=False)
    oq_in = nc.declare_dram_parameter("oq", [BS, D], f32, isOutput=False)
    oqv_in = nc.declare_dram_parameter("oqv", [BS, D], f32, isOutput=False)

    wx_in = nc.declare_dram_parameter("wx", [128, KD * INNER], bf16, isOutput=False)
    augx_in = nc.declare_dram_parameter("augx", [2, INNER], bf16, isOutput=False)
    wtv_in = nc.declare_dram_parameter("wtv", [128, KI * D], bf16, isOutput=False)
    wv_in = nc.declare_dram_parameter("wv", [128, KD * D], bf16, isOutput=False)
    wq1_in = nc.declare_dram_parameter("wq1", [128, 16 * 256], bf16, isOutput=False)
    augq1_in = nc.declare_dram_parameter("augq1", [2, 256], bf16, isOutput=False)
    wq2_in = nc.declare_dram_parameter("wq2", [128, 2 * 2048], bf16, isOutput=False)
    bq2r_in = nc.declare_dram_parameter("bq2r", [128, 2048], bf16, isOutput=False)
    bvr_in = nc.declare_dram_parameter("bvr", [128, D], bf16, isOutput=False)
    wk_in = nc.declare_dram_parameter("wk", [128, KD * 392], bf16, isOutput=False)
    augk_in = nc.declare_dram_parameter("augk", [2, 392], bf16, isOutput=False)
    wtn_in = nc.declare_dram_parameter("wtn", [128, KI * D], bf16, isOutput=False)

    xout = nc.declare_dram_parameter("xout", [BS, S, D], f32, isOutput=True)
    qout = nc.declare_dram_parameter("qout", [BS, D], f32, isOutput=True)
    newk = nc.declare_dram_parameter("newk", [BS, D], f32, isOutput=True)

    with tile.TileContext(nc) as tc, ExitStack() as ctx:
        wp = ctx.enter_context(tc.tile_pool(name="weights", bufs=1))
        persist = ctx.enter_context(tc.tile_pool(name="persist", bufs=1))
        xin_pool = ctx.enter_context(tc.tile_pool(name="xin", bufs=10))
        work = ctx.enter_context(tc.tile_pool(name="work", bufs=3))
        stats_pool = ctx.enter_context(tc.tile_pool(name="stats", bufs=8))
        out_pool = ctx.enter_context(tc.tile_pool(name="outs", bufs=2))
        once = ctx.enter_context(tc.tile_pool(name="once", bufs=1))
        # PSUM budget (8 banks): xt 1x2 + vt 1x2 + xv 1x2 + xo 2x1 = 8
        psum = ctx.enter_context(tc.tile_pool(name="psum", bufs=2, space="PSUM"))
        psum2 = ctx.enter_context(tc.tile_pool(name="psum2", bufs=2, space="PSUM"))
        psum_o = ctx.enter_context(tc.tile_pool(name="psum_o", bufs=1, space="PSUM"))

        # ---- constants ----
        ident = wp.tile([128, 128], bf16)
        make_identity(nc, ident)
        ones2 = wp.tile([2, 128], bf16)
        nc.vector.memset(ones2, 0.0)
        nc.vector.memset(ones2[0:1, :], 1.0)
        eps_t = wp.tile([128, 1], f32)
        nc.vector.memset(eps_t, EPS)

        def wtile(name, dram, shape):
            t = wp.tile(list(shape), bf16, tag=f"w_{name}")
            if len(shape) == 3:
                nc.sync.dma_start(out=t, in_=dram.rearrange("p (k n) -> p k n", k=shape[1]))
            else:
                nc.sync.dma_start(out=t, in_=dram[:, :])
            return t

        q_sb = persist.tile([BS, D], f32)
        oq_sb = persist.tile([BS, D], f32)
        oqv_sb = persist.tile([BS, D], f32)
        nc.sync.dma_start(out=q_sb, in_=q_in[:, :])
        nc.sync.dma_start(out=oq_sb, in_=oq_in[:, :])
        nc.sync.dma_start(out=oqv_sb, in_=oqv_in[:, :])
        # per-sample-stage weights first so sigk (which gates every tile)
        # is computable as early as possible; bulk tile-loop weights after.
        wq1 = wtile("wq1", wq1_in, (128, 16, 256))
        augq1 = wtile("augq1", augq1_in, (2, 256))
        wq2 = wtile("wq2", wq2_in, (128, 2, 2048))
        bq2r = wtile("bq2r", bq2r_in, (128, 2048))
        wv = wtile("wv", wv_in, (128, KD, D))
        bvr = wtile("bvr", bvr_in, (128, D))
        wk = wtile("wk", wk_in, (128, KD, 392))
        augk = wtile("augk", augk_in, (2, 392))
        wx = wtile("wx", wx_in, (128, KD, INNER))
        augx = wtile("augx", augx_in, (2, INNER))
        wtv = wtile("wtv", wtv_in, (128, KI, D))
        wtn = wtile("wtn", wtn_in, (128, KI, D))

        import concourse.bass as _bass

        def transpose_to_sbuf(src_bf16, width, copy_engine, tag, split=1):
            """PE-transpose [128, width] bf16 -> PSUM -> copy to SBUF.
            split>1 copies out in pieces so downstream matmuls start sooner."""
            tp = psum.tile([128, width], bf16, tag="xt" if width > 512 else "vt")
            for k in range(width // 128):
                nc.tensor.transpose(tp[:, k * 128:(k + 1) * 128],
                                    src_bf16[:, k * 128:(k + 1) * 128], ident)
            pool = once if tag.startswith("ps_") else work
            nb = 6 if tag == "xnt" else None
            ts = pool.tile([128, width], bf16, tag=tag, name=f"t_{tag}", bufs=nb)
            step = width // split
            for j in range(split):
                copy_engine(out=ts[:, j * step:(j + 1) * step],
                            in_=tp[:, j * step:(j + 1) * step])
            return ts

        def layer_norm_bf16(src_f32, out_tag):
            """bn_stats LN over free dim 1024 -> normalized bf16 tile."""
            v = src_f32.rearrange("p (g d) -> p g d", g=2)
            st = stats_pool.tile([128, 2, 6], f32, tag="bnst")
            for g in range(2):
                nc.vector.bn_stats(out=st[:, g, :], in_=v[:, g, :])
            mv = stats_pool.tile([128, 2], f32, tag="mv")
            nc.vector.bn_aggr(out=mv, in_=st)
            sd = stats_pool.tile([128, 1], f32, tag="sd")
            nc.scalar.activation(out=sd, in_=mv[:, 1:2], func=AF.Sqrt,
                                 bias=eps_t, scale=1.0)
            rstd = stats_pool.tile([128, 1], f32, tag="rstd")
            nc.vector.reciprocal(out=rstd, in_=sd)
            xn = work.tile([128, D], bf16, tag=out_tag)
            nc.vector.tensor_scalar(out=xn, in0=src_f32, scalar1=mv[:, 0:1],
                                    scalar2=rstd[:, 0:1],
                                    op0=ALU.subtract, op1=ALU.mult)
            return xn

        # ================= per-sample stage =================

        # qk = [q - oq, oq] in bf16
        qk1 = once.tile([BS, D], bf16, tag="ps_a")
        nc.vector.tensor_sub(qk1, q_sb, oq_sb)
        qk2 = once.tile([BS, D], bf16, tag="ps_b")
        nc.vector.tensor_copy(qk2, oq_sb)
        qk1T = transpose_to_sbuf(qk1, D, nc.scalar.copy, "ps_at")
        qk2T = transpose_to_sbuf(qk2, D, nc.scalar.copy, "ps_bt")

        # h1 = qk @ Wq1 + bq1  [BS, 256]
        h1_ps = psum2.tile([128, 256], f32, tag="xv")
        for k in range(8):
            nc.tensor.matmul(h1_ps, qk1T[:, k * 128:(k + 1) * 128], wq1[:, k, :],
                             start=(k == 0), stop=False)
        for k in range(8):
            nc.tensor.matmul(h1_ps, qk2T[:, k * 128:(k + 1) * 128], wq1[:, 8 + k, :],
                             start=False, stop=False)
        nc.tensor.matmul(h1_ps, ones2, augq1, start=False, stop=True)

        # silu(h1) = h1 * sigmoid(h1)
        sg1 = once.tile([BS, 256], f32, tag="ps_sg1")
        nc.scalar.activation(out=sg1, in_=h1_ps, func=AF.Sigmoid)
        hsil = once.tile([BS, 256], bf16, tag="ps_hsil")
        nc.vector.tensor_mul(hsil, h1_ps, sg1)
        hT = transpose_to_sbuf(hsil, 256, nc.scalar.copy, "ps_ht")

        # h2 = silu(h1) @ Wq2 (+ bq2 on evict), split into query/key halves
        sqk = []
        for half in range(2):
            h2_h = [psum_o.tile([128, 512], f32, tag=f"xo{n}", name=f"h2_h{n}") for n in range(2)]
            for n in range(2):
                for k in range(2):
                    nc.tensor.matmul(
                        h2_h[n],
                        hT[:, k * 128:(k + 1) * 128],
                        wq2[:, k, half * 1024 + n * 512: half * 1024 + (n + 1) * 512],
                        start=(k == 0), stop=(k == 1))
            th = once.tile([BS, D], f32, tag=f"ps_th{half}")
            for n in range(2):
                nc.vector.tensor_add(th[:, n * 512:(n + 1) * 512], h2_h[n],
                                     bq2r[:, half * 1024 + n * 512:half * 1024 + (n + 1) * 512])
            sig = persist.tile([BS, D], f32, tag=f"ps_sig{half}")
            nc.scalar.activation(out=sig, in_=th, func=AF.Sigmoid)
            sqk.append(sig)

        # val = oq_v @ Wv + bv
        oqv_bf = once.tile([BS, D], bf16, tag="ps_a")
        nc.vector.tensor_copy(oqv_bf, oqv_sb)
        oqvT = transpose_to_sbuf(oqv_bf, D, nc.scalar.copy, "ps_at")
        val_h = [psum_o.tile([128, 512], f32, tag=f"xo{n}", name=f"val_h{n}") for n in range(2)]
        for n in range(2):
            for k in range(KD):
                nc.tensor.matmul(val_h[n],
                                 oqvT[:, k * 128:(k + 1) * 128],
                                 wv[:, k, n * 512:(n + 1) * 512],
                                 start=(k == 0), stop=(k == KD - 1))
        val_sb = persist.tile([BS, D], f32)
        for n in range(2):
            nc.vector.tensor_add(val_sb[:, n * 512:(n + 1) * 512], val_h[n],
                                 bvr[:, n * 512:(n + 1) * 512])

        # new_q / new_k
        nq = persist.tile([BS, D], f32)
        nc.vector.tensor_mul(nq, sqk[0], val_sb)
        nc.vector.tensor_add(nq, nq, q_sb)
        nk = persist.tile([BS, D], f32)
        nc.vector.tensor_mul(nk, sqk[1], val_sb)
        nc.vector.tensor_add(nk, nk, oq_sb)
        nc.scalar.dma_start(out=newk[:, :], in_=nk)

        # kmap = LN(new_q) @ Wk' + c0k ; sigk = sigmoid(kmap)
        zq = layer_norm_bf16(nq, "ps_a")
        zqT = transpose_to_sbuf(zq, D, nc.scalar.copy, "ps_at")
        kmap_ps = psum2.tile([128, 392], f32, tag="xv")
        for k in range(KD):
            nc.tensor.matmul(kmap_ps, zqT[:, k * 128:(k + 1) * 128], wk[:, k, :],
                             start=(k == 0), stop=False)
        nc.tensor.matmul(kmap_ps, ones2, augk, start=False, stop=True)
        sigk = persist.tile([BS, H * S], bf16)
        nc.scalar.activation(out=sigk, in_=kmap_ps, func=AF.Sigmoid)

        # running sum of value over positions (for q_out)
        vacc = persist.tile([BS, INNER], f32)
        nc.vector.memset(vacc, 0.0)

        # ================= position-tile loop =================
        # Tiles are processed in pairs with matmul work batched per pair:
        # the PE sees long uninterrupted MATMUL bursts (keeps the HAM clock
        # gate at 8/8) instead of alternating transpose/matmul every tile.
        import os as _os
        _ntiles = int(_os.environ.get("K_NTILES", S))

        def tile_phase1(s):
            x_sb = xin_pool.tile([BS, D], f32, tag="x", name=f"x_{s}")
            nc.sync.dma_start(out=x_sb, in_=x_in[:, s, :])
            xn = layer_norm_bf16(x_sb, "xn")
            xnT = transpose_to_sbuf(xn, D, nc.scalar.copy, "xnt", split=2)
            return x_sb, xnT

        def tile_tox(xnT):
            xv_ps = psum2.tile([128, INNER], f32, tag="xv", name="xv_ps")
            for k in range(KD):
                nc.tensor.matmul(xv_ps, xnT[:, k * 128:(k + 1) * 128], wx[:, k, :],
                                 start=(k == 0), stop=False)
            nc.tensor.matmul(xv_ps, ones2, augx, start=False, stop=True)
            return xv_ps

        def tile_gate(s, xv_ps):
            gate_ap = _bass.AP(tensor=sigk.tensor, offset=sigk.offset + s,
                               ap=[sigk.ap[0], [S, H], [0, 64]])
            value = work.tile([BS, INNER], bf16, tag="value", name=f"value_{s}")
            nc.vector.tensor_mul(value, xv_ps, gate_ap)
            nc.gpsimd.tensor_add(vacc, vacc, value)
            return value

        def tile_tov(s, x_sb, vT):
            xo_h = [psum_o.tile([128, 512], f32, tag=f"xo{n}", name=f"xo_h{n}") for n in range(2)]
            xo_sb = out_pool.tile([BS, D], f32, tag="xo_sb", name=f"xo_sb_{s}")
            for n in range(2):
                for k in range(KI):
                    nc.tensor.matmul(xo_h[n],
                                     vT[:, k * 128:(k + 1) * 128],
                                     wtv[:, k, n * 512:(n + 1) * 512],
                                     start=(k == 0), stop=(k == KI - 1))
                nc.scalar.copy(out=xo_sb[:, n * 512:(n + 1) * 512], in_=xo_h[n])
            nc.gpsimd.tensor_add(xo_sb, xo_sb, x_sb)
            nc.scalar.dma_start(out=xout[:, s, :], in_=xo_sb)

        def tile_phase2(pair, p1):
            xvs = [tile_tox(xnT) for (_, xnT) in p1]
            vals = [tile_gate(s, xv) for s, xv in zip(pair, xvs)]
            vTs = [transpose_to_sbuf(v, INNER, nc.vector.tensor_copy, "vt")
                   for v in vals]
            for (x_sb, _), s, vT in zip(p1, pair, vTs):
                tile_tov(s, x_sb, vT)

        # one pair of lookahead: phase1(p+1) is emitted before phase2(p) so
        # the next pair's LN/transposes are scheduled ahead of this pair's
        # evictions on the ACT/DVE queues.
        pending = None
        for s0 in range(0, _ntiles, 2):
            pair = [s for s in (s0, s0 + 1) if s < _ntiles]
            p1 = [tile_phase1(s) for s in pair]
            if pending is not None:
                tile_phase2(*pending)
            pending = (pair, p1)
        if pending is not None:
            tile_phase2(*pending)

        # ================= q_out tail =================
        vacc_bf = work.tile([BS, INNER], bf16, tag="value")
        nc.vector.tensor_copy(vacc_bf, vacc)
        vaccT = transpose_to_sbuf(vacc_bf, INNER, nc.vector.tensor_copy, "vt")
        qo_h = [psum_o.tile([128, 512], f32, tag=f"xo{n}", name=f"qo_h{n}") for n in range(2)]
        qo_sb = persist.tile([BS, D], f32, tag="qo_sb")
        for n in range(2):
            for k in range(KI):
                nc.tensor.matmul(qo_h[n],
                                 vaccT[:, k * 128:(k + 1) * 128],
                                 wtn[:, k, n * 512:(n + 1) * 512],
                                 start=(k == 0), stop=(k == KI - 1))
            nc.vector.tensor_add(qo_sb[:, n * 512:(n + 1) * 512], qo_h[n],
                                 q_sb[:, n * 512:(n + 1) * 512])
        nc.scalar.dma_start(out=qout[:, :], in_=qo_sb)

    nc.finalize()
    return nc


def _pack(w, kchunks):
    """[K*128, N] -> [128, kchunks*N] bf16 with k-chunk-major free dim."""
    w = np.asarray(w, np.float32)
    n = w.shape[1]
    return np.ascontiguousarray(
        w.astype(BF).reshape(kchunks, 128, n).transpose(1, 0, 2)
    ).reshape(128, kchunks * n)


def kernel(**inputs):
    global _CACHED_NC, LAST_RESULT
    from concourse.bass_utils import run_bass_kernel_spmd

    f = lambda name: np.asarray(inputs[name], np.float32)
    x = f("x")
    q = f("q").reshape(B, D)
    oq = f("oq").reshape(B, D)
    oqv = f("oq_v").reshape(B, D)

    gx, bxln, Wx, bx = f("gx"), f("bx_ln"), f("Wx"), f("bx")
    gk, bkln, Wk, bk = f("gk"), f("bk_ln"), f("Wk"), f("bk")
    Wq1, bq1 = f("Wq1"), f("bq1")
    Wq2, bq2 = f("Wq2"), f("bq2")
    Wv, bv = f("Wv"), f("bv")
    Wtv, Wtn = f("Wtv"), f("Wtn")

    wxf = gx[:, None] * Wx
    c0x = bxln @ Wx + bx
    wkf = gk[:, None] * Wk
    c0k = bkln @ Wk + bk

    def aug(row0, n):
        a = np.zeros((2, n), dtype=BF)
        a[0] = row0.astype(BF)
        return a

    weights = dict(
        wx=_pack(wxf, KD), augx=aug(c0x, INNER),
        wtv=_pack(Wtv, KI), wv=_pack(Wv, KD),
        wq1=_pack(Wq1, 16), augq1=aug(bq1, 256),
        wq2=_pack(Wq2, 2),
        bq2r=np.ascontiguousarray(np.tile(bq2.astype(BF), (128, 1))),
        bvr=np.ascontiguousarray(np.tile(bv.astype(BF), (128, 1))),
        wk=_pack(wkf, KD), augk=aug(c0k, 392),
        wtn=_pack(Wtn / np.float32(S), KI),
    )

    if _CACHED_NC is None:
        _CACHED_NC = _build_nc()
    nc = _CACHED_NC

    in_maps = []
    for i in range(NCORES):
        sl = slice(i * BS, (i + 1) * BS)
        in_maps.append(dict(
            x=np.ascontiguousarray(x[sl]),
            q=np.ascontiguousarray(q[sl]),
            oq=np.ascontiguousarray(oq[sl]),
            oqv=np.ascontiguousarray(oqv[sl]),
            **weights,
        ))

    LAST_RESULT = run_bass_kernel_spmd(nc, in_maps, list(range(NCORES)))
    res = LAST_RESULT.results

    x_out = np.concatenate([r["xout"] for r in res], axis=0)
    q_out = np.concatenate([r["qout"] for r in res], axis=0).reshape(B, 1, D)
    new_k = np.concatenate([r["newk"] for r in res], axis=0).reshape(B, 1, D)
    return (np.asarray(x_out, np.float32), np.asarray(q_out, np.float32),
            np.asarray(new_k, np.float32))
